# revision 32
# baseline (speedup 1.0000x reference)
"""Trainium2 Bass kernel for nn_Assessor voxel-grid scoring.

Input : voxel_grid [4,1,128,128,128] f32 (uniform [0,1)).
Output: (overhang, surface_score) — two f32 scalars, matching reference.py.

Strategy
--------
8 cores = 4 batches x 2 z-halves. Each core gets its half's 64 z-slices plus a
10-slice redundant halo (ITERS=10 flood-fill steps propagate 1 voxel/step), so
no inter-core communication is needed at all. Odd cores (upper z-half) receive
their shard z-FLIPPED so every core runs an identical SPMD program (own region
is always buffer slices [1,65), halo [65,75), and the globally-z-border slice
is always buffer slice 1). The 6-neighbor flood-fill stencil runs on the
TensorEngine as 5 accumulating matmuls per 512-column chunk (h+-1 via a
tridiagonal shift matrix, z/w shifts via identity matmuls at shifted moving-
operand offsets), sigmoid(20c-10) on ScalarE straight out of PSUM, and
max+mask*not_solid on VectorE in bf16. Flood iteration 0 is computed
analytically (mask1 = sigmoid(-10)*not_solid except the 7 seed-neighborhood
cells, patched via tiny host-supplied DMAs). All other terms (overhang
support product T2, surface border-weighted sums, occupancy) are exact f32
reductions; T2's chunks are interleaved into the flood loop to fill engine
gaps. Each core emits 16 per-partition partial-sum columns; the host reduces
partitions and combines cores into the two scalars.

Layout per core: [h=128 partitions, (z,w) free] with W padded 128->130 and
z padded 74->76 by zero columns so all stencil shifts are plain flat-offset
reads with correct zero boundary conditions.
"""

import numpy as np
import ml_dtypes

import concourse.bacc as bacc
import concourse.mybir as mybir
from concourse.tile import TileContext
from concourse.bass_utils import run_bass_kernel_spmd

P = 128          # partitions = H
W = 128
WP = 130         # padded W
NZ = 74          # data z-slices per core (64 own + 10 halo)
ZP = 76          # padded z (zero slice each side)
COLS = ZP * WP   # 9880 flat free columns
ITERS = 10
T2_ON = True     # build-time knob for phase timing experiments
CH = 512         # chunk columns (one PSUM bank of f32)

INT0 = WP                 # first interior col (buf z=1)
INTN = NZ * WP            # interior col count (9620)

# T2 (overhang support product): pairs (curr=buf z b, below=b-1) for b in [2,65)
T2_BELOW0 = WP            # below cols [130, 64*130)
T2_BELOWN = 63 * WP       # 8190
T2_STEP = CH - 2          # p-chunks overlap by 2 for the +-1 w-window

SIG_NEG = float(1.0 / (1.0 + np.exp(10.0)))   # sigmoid(-10)
SIG_POS = float(1.0 / (1.0 + np.exp(-10.0)))  # sigmoid(+10)

F32 = mybir.dt.float32
BF16 = mybir.dt.bfloat16


def _build_nc():
    nc = bacc.Bacc(None, target_bir_lowering=False)
    x = nc.dram_tensor("x", [P, COLS], F32, kind="ExternalInput")
    cb = nc.dram_tensor("cb", [P, 256], BF16, kind="ExternalInput")   # [Sh | I]
    sf = nc.dram_tensor("sf", [P, 128], F32, kind="ExternalInput")    # Sh + I
    vv = nc.dram_tensor("vv", [P, 4], F32, kind="ExternalInput")      # ones,dH
    sp = nc.dram_tensor("sp", [P, 8], BF16, kind="ExternalInput")     # patches
    out = nc.dram_tensor("out", [P, 16], F32, kind="ExternalOutput")

    alu = mybir.AluOpType
    sig = mybir.ActivationFunctionType.Sigmoid
    cpy = mybir.ActivationFunctionType.Copy
    with TileContext(nc) as tc:
        with (
            tc.tile_pool(name="singles", bufs=1) as sg,
            tc.tile_pool(name="psum", bufs=6, space="PSUM") as pp,
            tc.tile_pool(name="work", bufs=4) as wk,
        ):
            xs = sg.tile([P, COLS], F32)
            ns = sg.tile([P, COLS], BF16)
            m_a = sg.tile([P, COLS], BF16)
            m_b = sg.tile([P, COLS], BF16)
            cbs = sg.tile([P, 256], BF16)
            sfs = sg.tile([P, 128], F32)
            vvs = sg.tile([P, 4], F32)
            red = sg.tile([P, 16], F32)     # per-partition partial columns
            acc = sg.tile([P, 4], F32)      # small temporaries
            junk = sg.tile([P, CH], F32)
            accT2 = sg.tile([P, 20], F32)
            biasc = sg.tile([P, 1], F32)

            ZB_SPLIT = [1, 17, 33, 49, 65, 75]
            xs_cuts = [0, 17 * WP, 33 * WP, 49 * WP, 65 * WP, COLS]
            for a, b in zip(xs_cuts[:-1], xs_cuts[1:]):
                nc.gpsimd.dma_start(out=xs[:, a:b], in_=x[:, a:b])
            nc.gpsimd.dma_start(out=cbs[:, :], in_=cb[:, :])
            nc.gpsimd.dma_start(out=sfs[:, :], in_=sf[:, :])
            nc.gpsimd.dma_start(out=vvs[:, :], in_=vv[:, :])

            x3 = xs[:, :].rearrange("p (a b) -> p a b", b=WP)
            ns3 = ns[:, :].rearrange("p (a b) -> p a b", b=WP)
            ma3 = m_a[:, :].rearrange("p (a b) -> p a b", b=WP)

            sh_b = cbs[:, 0:128]
            id_b = cbs[:, 128:256]

            nc.vector.memset(biasc[:, :], -10.0)
            nc.vector.memset(red[:, :], 0.0)
            nc.vector.memset(accT2[:, :], 0.0)

            # ---- ns = 1-x on data cols; pads stay zero (memset pads only) --
            # own-region pass carries accum -> R0 = 64*128 - sum(1-x)
            nc.vector.memset(ns3[:, 0:1, :], 0.0)            # z pad lo
            nc.vector.memset(ns3[:, 75:76, :], 0.0)          # z pad hi
            nc.vector.memset(ns3[:, 1:75, 0:1], 0.0)         # w pad lo
            nc.vector.memset(ns3[:, 1:75, 129:130], 0.0)     # w pad hi
            # ns = 1-x on ScalarE (Copy: out = -1*x + 1), in z-pieces so each
            # starts as soon as its DMA piece lands; own pieces accumulate
            # sum(1-x) -> R0 = 8192 - sum(acc).  m_b = sigmoid(-10)*ns fill
            # follows piece-by-piece so flood iter 1 can start early.
            for i in range(4):
                za, zend = ZB_SPLIT[i], ZB_SPLIT[i + 1]
                nc.scalar.activation(ns3[:, za:zend, 1:129],
                                     x3[:, za:zend, 1:129],
                                     cpy, scale=-1.0, bias=1.0,
                                     accum_out=acc[:, i:i + 1])
                nc.scalar.activation(m_b[:, xs_cuts[i]:xs_cuts[i + 1]],
                                     ns[:, xs_cuts[i]:xs_cuts[i + 1]],
                                     cpy, scale=SIG_NEG, bias=0.0)
            nc.scalar.activation(ns3[:, 65:75, 1:129], x3[:, 65:75, 1:129],
                                 cpy, scale=-1.0, bias=1.0)
            nc.scalar.activation(m_b[:, xs_cuts[4]:], ns[:, xs_cuts[4]:],
                                 cpy, scale=SIG_NEG, bias=0.0)

            # ---------- flood-fill iteration 0 seed patches ----------------
            nc.gpsimd.dma_start(out=m_b[4:5, 786:787], in_=sp[4:5, 0:1])
            nc.gpsimd.dma_start(out=m_b[5:6, 656:657], in_=sp[5:6, 1:2])
            nc.gpsimd.dma_start(out=m_b[5:6, 785:788], in_=sp[5:6, 2:5])
            nc.gpsimd.dma_start(out=m_b[5:6, 916:917], in_=sp[5:6, 5:6])
            nc.gpsimd.dma_start(out=m_b[6:7, 786:787], in_=sp[6:7, 6:7])
            # m_a: only pads must be zero (interior fully written each iter)
            nc.vector.memset(ma3[:, 0:1, :], 0.0)
            nc.vector.memset(ma3[:, 75:76, :], 0.0)
            nc.vector.memset(ma3[:, 1:75, 0:1], 0.0)
            nc.vector.memset(ma3[:, 1:75, 129:130], 0.0)

            # ---------- T2 chunk emitter (interleaved into flood loop) ------
            t2_state = {"pc0": T2_BELOW0 - 1, "i": 0}
            t2_end = T2_BELOW0 + T2_BELOWN + 1 if T2_ON else T2_BELOW0 - 1

            def emit_t2_chunk():
                pc0 = t2_state["pc0"]
                if pc0 + 2 >= t2_end:
                    return
                w = min(CH, t2_end - pc0)
                n = w - 2
                ps = pp.tile([P, CH], F32, tag="ps")
                nc.tensor.matmul(ps[:, :w], sfs[:, :], xs[:, pc0:pc0 + w],
                                 start=True, stop=True)
                u = wk.tile([P, CH], F32, tag="u")
                nc.vector.tensor_tensor(
                    u[:, :n], xs[:, pc0 + 130:pc0 + 130 + n],
                    xs[:, pc0 + 132:pc0 + 132 + n], op=alu.add)
                nc.vector.tensor_tensor(
                    u[:, :n], u[:, :n],
                    xs[:, pc0 + 131:pc0 + 131 + n], op=alu.add)
                tmp = wk.tile([P, CH], F32, tag="tmp")
                nc.vector.tensor_tensor(
                    tmp[:, :n], ps[:, 1:1 + n], u[:, :n], op=alu.mult)
                i = t2_state["i"]
                nc.scalar.activation(junk[:, :n], tmp[:, :n], cpy, scale=1.0,
                                     accum_out=accT2[:, i:i + 1])
                t2_state["i"] = i + 1
                t2_state["pc0"] = pc0 + T2_STEP

            # ---------- flood fill iterations 1..9 --------------------------
            for it in range(1, ITERS):
                src = m_a if it % 2 == 0 else m_b
                dst = m_b if it % 2 == 0 else m_a
                c0 = INT0
                ci = 0
                while c0 < INT0 + INTN:
                    w = min(CH, INT0 + INTN - c0)
                    ps = pp.tile([P, CH], F32, tag="ps")
                    nc.tensor.matmul(ps[:, :w], sh_b, src[:, c0:c0 + w],
                                     start=True, stop=False)
                    nc.tensor.matmul(ps[:, :w], id_b,
                                     src[:, c0 - WP:c0 - WP + w],
                                     start=False, stop=False)
                    nc.tensor.matmul(ps[:, :w], id_b,
                                     src[:, c0 + WP:c0 + WP + w],
                                     start=False, stop=False)
                    nc.tensor.matmul(ps[:, :w], id_b,
                                     src[:, c0 - 1:c0 - 1 + w],
                                     start=False, stop=False)
                    nc.tensor.matmul(ps[:, :w], id_b,
                                     src[:, c0 + 1:c0 + 1 + w],
                                     start=False, stop=True)
                    av = wk.tile([P, CH], BF16, tag="act")
                    nc.scalar.activation(av[:, :w], ps[:, :w], sig,
                                         bias=biasc[:, :], scale=20.0)
                    mx = wk.tile([P, CH], BF16, tag="mx")
                    nc.vector.tensor_tensor(mx[:, :w], src[:, c0:c0 + w],
                                            av[:, :w], op=alu.max)
                    nc.vector.tensor_tensor(dst[:, c0:c0 + w], mx[:, :w],
                                            ns[:, c0:c0 + w], op=alu.mult)
                    c0 += CH
                    ci += 1
                    if ci % 9 == 0:
                        emit_t2_chunk()

            while t2_state["pc0"] + 2 < t2_end:
                emit_t2_chunk()
            if T2_ON:
                nc.vector.tensor_reduce(
                    red[:, 8:9], accT2[:, 0:t2_state["i"]],
                    axis=mybir.AxisListType.X, op=alu.add)

            fin = m_a if (ITERS - 1) % 2 == 1 else m_b
            fin3 = fin[:, :].rearrange("p (a b) -> p a b", b=WP)
            jrb = sg.tile([P, 64 * 128], BF16)
            nc.scalar.activation(jrb[:, :], fin3[:, 1:65, 1:129], cpy,
                                 scale=1.0, bias=0.0,
                                 accum_out=red[:, 9:10])

            # ---------- small exact reductions (emitted last: fill the tail)
            nc.vector.tensor_tensor(acc[:, 0:1], acc[:, 0:1], acc[:, 1:2],
                                    op=alu.add)
            nc.vector.tensor_tensor(acc[:, 2:3], acc[:, 2:3], acc[:, 3:4],
                                    op=alu.add)
            nc.vector.tensor_tensor(acc[:, 0:1], acc[:, 0:1], acc[:, 2:3],
                                    op=alu.add)
            nc.vector.tensor_scalar(
                out=red[:, 0:1], in0=acc[:, 0:1],
                scalar1=-1.0, scalar2=float(64 * 128),
                op0=alu.mult, op1=alu.add)
            nc.vector.tensor_reduce(
                red[:, 2:3], x3[:, 1:65, 1:2], axis=mybir.AxisListType.XY,
                op=alu.add)
            nc.vector.tensor_reduce(
                acc[:, 2:3], x3[:, 1:65, 128:129], axis=mybir.AxisListType.XY,
                op=alu.add)
            nc.vector.tensor_tensor(
                red[:, 2:3], red[:, 2:3], acc[:, 2:3], op=alu.add)
            nc.vector.tensor_reduce(
                red[:, 4:5], x3[:, 1:2, 1:129], axis=mybir.AxisListType.X,
                op=alu.add)
            nc.vector.tensor_tensor(
                red[:, 6:7], x3[:, 1:2, 1:2], x3[:, 1:2, 128:129], op=alu.add)
            for src_c, dst_c in ((0, 1), (2, 3), (4, 5), (6, 7)):
                nc.vector.tensor_tensor(
                    red[:, dst_c:dst_c + 1], red[:, src_c:src_c + 1],
                    vvs[:, 1:2], op=alu.mult)

            # ---------- output: per-partition partials, host sums ----------
            nc.gpsimd.dma_start(out=out[:, :], in_=red[:, :])
    nc.finalize()
    return nc


_NC_CACHE = {}


def _get_nc():
    if "nc" not in _NC_CACHE:
        _NC_CACHE["nc"] = _build_nc()
    return _NC_CACHE["nc"]


def _host_inputs(voxel_grid):
    xg = np.asarray(voxel_grid, dtype=np.float32)  # [4,1,128,128,128]
    sh = np.zeros((128, 128), np.float32)
    for i in range(128):
        if i > 0:
            sh[i - 1, i] = 1.0
        if i < 127:
            sh[i + 1, i] = 1.0
    eye = np.eye(128, dtype=np.float32)
    cb = np.concatenate([sh, eye], axis=1).astype(ml_dtypes.bfloat16)
    sf = (sh + eye).astype(np.float32)
    vv = np.zeros((P, 4), np.float32)
    vv[:, 0] = 1.0
    vv[0, 1] = 1.0
    vv[127, 1] = 1.0

    in_maps = []
    for k in range(8):
        b, half = k // 2, k % 2
        if half == 0:
            sub = xg[b, 0, 0:NZ]                 # [74,128,128]
        else:
            sub = xg[b, 0, 128 - NZ:128][::-1]   # z-flipped upper half
        buf = np.zeros((P, ZP, WP), np.float32)
        buf[:, 1:1 + NZ, 1:1 + W] = np.ascontiguousarray(sub.transpose(1, 0, 2))
        # iteration-0 patch values around the seed (z=5,h=5,w=5 -> buf z=6)
        f_seed = 1.0 if half == 0 else SIG_NEG
        f_nb = SIG_POS if half == 0 else SIG_NEG
        spv = np.zeros((P, 8), np.float64)
        spv[4, 0] = f_nb * (1.0 - buf[4, 6, 6])
        spv[5, 1] = f_nb * (1.0 - buf[5, 5, 6])
        spv[5, 2] = f_nb * (1.0 - buf[5, 6, 5])
        spv[5, 3] = f_seed * (1.0 - buf[5, 6, 6])
        spv[5, 4] = f_nb * (1.0 - buf[5, 6, 7])
        spv[5, 5] = f_nb * (1.0 - buf[5, 7, 6])
        spv[6, 6] = f_nb * (1.0 - buf[6, 6, 6])
        in_maps.append({
            "x": buf.reshape(P, COLS),
            "cb": cb,
            "sf": sf,
            "vv": vv,
            "sp": spv.astype(ml_dtypes.bfloat16),
        })
    return in_maps


def _host_combine(voxel_grid, results):
    xg = np.asarray(voxel_grid, dtype=np.float64)
    B, _, D, H, Wd = xg.shape
    N = float(B * D * H * Wd)
    vals = np.stack([np.asarray(r["out"], np.float64).reshape(P, 16).sum(axis=0)
                     for r in results])  # [8,16]
    S0, Fh, Fw, Ehw, Fz, Ezh, Ezw, C, T2, R = (vals[:, i] for i in range(10))

    # missing T2 term: z=64 pairs with z=63 (global), per batch, on host
    t2_fix = 0.0
    for b in range(B):
        below = np.pad(xg[b, 0, 63], 1)
        win = np.zeros((H, Wd))
        for dh in range(3):
            for dw in range(3):
                win += below[dh:dh + H, dw:dw + Wd]
        t2_fix += float((xg[b, 0, 64] * win).sum())

    S0t = S0.sum()
    T1 = S0t - Fz[0::2].sum()          # subtract global z=0 slices (low cores)
    T2t = T2.sum() + t2_fix
    overhang = T1 / N - T2t / (9.0 * N)

    surface_sum = (27.0 * S0t - 9.0 * (Fz.sum() + Fh.sum() + Fw.sum())
                   + 3.0 * (Ezh.sum() + Ezw.sum() + Ehw.sum()) - C.sum())
    surface_term = surface_sum / 27.0 / N
    occupancy = S0t / N
    occ_pen = 10.0 * (occupancy - 0.5) ** 2
    resin = 100.0 * R.sum() / N
    surface_score = surface_term - resin - occ_pen
    return (np.float32(overhang), np.float32(surface_score))


def kernel(voxel_grid, _trace=False):
    nc = _get_nc()
    in_maps = _host_inputs(voxel_grid)
    res = run_bass_kernel_spmd(nc, in_maps, core_ids=list(range(8)),
                               trace=_trace)
    if _trace:
        _NC_CACHE["last_res"] = res
    return _host_combine(voxel_grid, res.results)


if __name__ == "__main__":
    rng = np.random.default_rng(0)
    vg = rng.random((4, 1, 128, 128, 128), dtype=np.float32)
    print(kernel(vg))
